# revision 3
# baseline (speedup 1.0000x reference)
"""GPT-2-ish forward (B=4, T=1024, D=768, H=12, L=2, V=50257) on 8 trn2 cores.

Sharding: core pair (2b, 2b+1) both run the full trunk for batch b
(replicated, zero collectives); lm_head is vocab-split within the pair
(each core does 25600 of the host-padded 51200 vocab columns).

On-device layout: activations transposed [features, tokens]. Attention
uses attT [keys, q] as the stationary matmul operand with a ones-column
appended to V so the softmax denominator lands in the free dim of the
(att @ V_aug) output; normalization is then a tensor_scalar_mul.
LayerNorm stats via ones-vector matmuls (contraction over partitions);
(g*rstd) / (b - g*mean*rstd) broadcasts built as rank-1 matmuls in PSUM.
All matmuls bf16 with fp32 PSUM accumulation; residual stream fp32 in
SBUF; logits evicted fp16 and upcast on host.
"""

import numpy as np
import ml_dtypes
from contextlib import ExitStack

import concourse.bass as bass
from concourse import bacc
import concourse.mybir as mybir
import concourse.tile as tile
from concourse.bass_utils import run_bass_kernel_spmd
from concourse.masks import make_identity

BF16 = mybir.dt.bfloat16
F32 = mybir.dt.float32
F16 = mybir.dt.float16
AF = mybir.ActivationFunctionType
ALU = mybir.AluOpType

V = 50257
VPAD = 51200          # 2 * 25600
VSH = VPAD // 2       # per-core vocab shard
D = 768
H = 12
HD = 64
L = 2
T = 1024
B = 4
EPS = 1e-5
NKT = D // 128        # 6 k-tiles over D
NQC = T // 512        # 2 q-chunks
NTT = T // 128        # 8 token-tiles
NVC = VSH // 512      # 50 lm vocab chunks per core

TRACE = False
LAST_RESULT = None

_SINGLES = {}


def _ln_phase(tc, nc, tag, xt, g_d, b_d, out_tiles, small, scratch):
    """LayerNorm over partition dim (features) of xt (6 fp32 [128,1024] tiles).
    g_d/b_d: [768] bf16 DRAM APs. Writes bf16 out_tiles (6 x [128,1024])."""
    ones_bf = _SINGLES["ones_bf"]
    ones_row = _SINGLES["ones_row"]

    g_bf = small.tile([1, D], BF16, tag="g_bf", name="g_bf")
    b_bf = small.tile([1, D], BF16, tag="b_bf", name="b_bf")
    nc.sync.dma_start(g_bf, g_d.rearrange("(o d) -> o d", o=1))
    nc.sync.dma_start(b_bf, b_d.rearrange("(o d) -> o d", o=1))
    rstd_bf = small.tile([1, T], BF16, tag="rstd_bf", name="rstd_bf")
    nmr_bf = small.tile([1, T], BF16, tag="nmr_bf", name="nmr_bf")
    eps_sb = small.tile([1, 1], F32, tag="eps_sb", name="eps_sb")
    nc.vector.memset(eps_sb, EPS)

    with tc.tile_pool(name=f"stps_{tag}", bufs=1, space="PSUM") as stats_ps, \
         tc.tile_pool(name=f"abps_{tag}", bufs=2, space="PSUM") as ab_ps:
        for c in range(NQC):
            s1 = stats_ps.tile([1, 512], F32, tag="s1", name="s1")
            s2 = stats_ps.tile([1, 512], F32, tag="s2", name="s2")
            for kt in range(NKT):
                xbf = scratch.tile([128, 512], BF16, tag="xbf", name="xbf")
                sq = scratch.tile([128, 512], BF16, tag="sq", name="sq")
                xs = xt[kt][:, c * 512:(c + 1) * 512]
                nc.vector.tensor_copy(xbf, xs)
                nc.vector.tensor_mul(sq, xs, xs)
                nc.tensor.matmul(s1, ones_bf, xbf,
                                 start=(kt == 0), stop=(kt == NKT - 1))
                nc.tensor.matmul(s2, ones_bf, sq,
                                 start=(kt == 0), stop=(kt == NKT - 1))
            # mean = s1/D ; var = s2/D - mean^2 ; rstd = 1/sqrt(var+eps)
            mean = small.tile([1, 512], F32, tag="mean", name="mean")
            var = small.tile([1, 512], F32, tag="var", name="var")
            rstd = small.tile([1, 512], F32, tag="rstd", name="rstd")
            nc.vector.tensor_scalar_mul(mean, s1, 1.0 / D)
            nc.vector.tensor_mul(var, mean, mean)
            nc.vector.scalar_tensor_tensor(var, s2, 1.0 / D, var,
                                           op0=ALU.mult, op1=ALU.subtract)
            nc.scalar.activation(var, var, AF.Sqrt, bias=eps_sb)
            nc.vector.reciprocal(rstd, var)
            nc.vector.tensor_copy(rstd_bf[:, c * 512:(c + 1) * 512], rstd)
            # nmr = -mean*rstd
            nc.vector.scalar_tensor_tensor(var, mean, -1.0, rstd,
                                           op0=ALU.mult, op1=ALU.mult)
            nc.vector.tensor_copy(nmr_bf[:, c * 512:(c + 1) * 512], var)

        for kt in range(NKT):
            gs = g_bf[0:1, kt * 128:(kt + 1) * 128]
            bs = b_bf[0:1, kt * 128:(kt + 1) * 128]
            for c in range(NQC):
                cs = slice(c * 512, (c + 1) * 512)
                a_ps = ab_ps.tile([128, 512], F32, tag="a_ps", name="a_ps")
                b_ps = ab_ps.tile([128, 512], F32, tag="b_ps", name="b_ps")
                nc.tensor.matmul(a_ps, gs, rstd_bf[:, cs], start=True, stop=True)
                nc.tensor.matmul(b_ps, gs, nmr_bf[:, cs], start=True, stop=False)
                nc.tensor.matmul(b_ps, bs, ones_row[:, 0:512],
                                 start=False, stop=True)
                tmp = scratch.tile([128, 512], F32, tag="lntmp", name="lntmp")
                nc.vector.tensor_mul(tmp, xt[kt][:, cs], a_ps)
                nc.vector.tensor_add(out_tiles[kt][:, cs], tmp, b_ps)


def build_bass():
    nc = bacc.Bacc(None, target_bir_lowering=False)
    # ---- DRAM I/O (per-core shard views) ----
    xT_d = nc.dram_tensor("xT", [D, T], F32, kind="ExternalInput")
    qkw_d = nc.dram_tensor("qkw", [L, D, 2 * D], BF16, kind="ExternalInput")
    vw_d = nc.dram_tensor("vw", [L, D, D], BF16, kind="ExternalInput")
    pw_d = nc.dram_tensor("pw", [L, D, D], BF16, kind="ExternalInput")
    fcw_d = nc.dram_tensor("fcw", [L, D, 4 * D], BF16, kind="ExternalInput")
    fc2w_d = nc.dram_tensor("fc2w", [L, 4 * D, D], BF16, kind="ExternalInput")
    qkb_d = nc.dram_tensor("qkb", [L, 2 * D], F32, kind="ExternalInput")
    vb_d = nc.dram_tensor("vb", [L, D], BF16, kind="ExternalInput")
    pb_d = nc.dram_tensor("pb", [L, D], F32, kind="ExternalInput")
    fcb_d = nc.dram_tensor("fcb", [L, 4 * D], F32, kind="ExternalInput")
    fc2b_d = nc.dram_tensor("fc2b", [L, D], F32, kind="ExternalInput")
    ln_d = nc.dram_tensor("lnp", [L, 4, D], BF16, kind="ExternalInput")  # g1,b1,g2,b2
    lnf_d = nc.dram_tensor("lnf", [2, D], BF16, kind="ExternalInput")
    mask_d = nc.dram_tensor("mask", [4, 128, 512], BF16, kind="ExternalInput")
    lmw_d = nc.dram_tensor("lmw", [D, VSH], BF16, kind="ExternalInput")
    out_d = nc.dram_tensor("out", [T, VSH], F16, kind="ExternalOutput")

    with tile.TileContext(nc) as tc, ExitStack() as octx:
        singles = octx.enter_context(tc.tile_pool(name="singles", bufs=1))
        resid = octx.enter_context(tc.tile_pool(name="resid", bufs=1))

        # constants
        ones_bf = singles.tile([128, 1], BF16)
        nc.vector.memset(ones_bf, 1.0)
        ones_row = singles.tile([1, 512], BF16)
        nc.vector.memset(ones_row, 1.0)
        ident = singles.tile([128, 128], BF16)
        make_identity(nc, ident)
        _SINGLES["ones_bf"] = ones_bf
        _SINGLES["ones_row"] = ones_row

        mask_sb = singles.tile([128, 4, 512], BF16)
        nc.sync.dma_start(mask_sb, mask_d.rearrange("j p q -> p j q"))

        # residual stream, fp32, resident
        xt = [resid.tile([128, T], F32, tag=f"xt{i}", name=f"xt{i}") for i in range(NKT)]
        for kt in range(NKT):
            nc.sync.dma_start(xt[kt], xT_d[kt * 128:(kt + 1) * 128, :])

        for l in range(L):
            with ExitStack() as lctx:
                lnpool = lctx.enter_context(tc.tile_pool(name=f"ln{l}", bufs=1))
                wpool = lctx.enter_context(tc.tile_pool(name=f"w{l}", bufs=3))
                biasp = lctx.enter_context(tc.tile_pool(name=f"bias{l}", bufs=1))
                small = lctx.enter_context(tc.tile_pool(name=f"small{l}", bufs=2))
                scratch = lctx.enter_context(tc.tile_pool(name=f"scr{l}", bufs=3))

                qkb_sb = biasp.tile([128, 12], F32)
                nc.sync.dma_start(qkb_sb, qkb_d[l].rearrange("(t p) -> p t", p=128))
                vbbf_sb = biasp.tile([1, D], BF16)
                nc.sync.dma_start(vbbf_sb, vb_d[l].rearrange("(o d) -> o d", o=1))
                pb_sb = biasp.tile([128, 6], F32)
                nc.sync.dma_start(pb_sb, pb_d[l].rearrange("(t p) -> p t", p=128))
                fcb_sb = biasp.tile([128, 24], F32)
                nc.sync.dma_start(fcb_sb, fcb_d[l].rearrange("(t p) -> p t", p=128))
                fc2b_sb = biasp.tile([128, 6], F32)
                nc.sync.dma_start(fc2b_sb, fc2b_d[l].rearrange("(t p) -> p t", p=128))

                # ---------- LN1 ----------
                h_bf = [lnpool.tile([128, T], BF16, tag=f"hbf{i}", name=f"hbf{i}")
                        for i in range(NKT)]
                _ln_phase(tc, nc, f"l{l}a", xt, ln_d[l][0], ln_d[l][1],
                          h_bf, small, scratch)

                # ---------- qkT = (qk_w).T @ h  [1536, 1024] bf16 ----------
                qk_sb = [lnpool.tile([128, T], BF16, tag=f"qk{i}", name=f"qk{i}")
                         for i in range(12)]
                with tc.tile_pool(name=f"qkps{l}", bufs=3, space="PSUM") as qkps:
                    for f in range(12):
                        wt = wpool.tile([128, NKT, 128], BF16, tag="qkw_t", name="qkw_t")
                        nc.sync.dma_start(
                            wt, qkw_d[l][:, f * 128:(f + 1) * 128]
                            .rearrange("(t p) f -> p t f", p=128))
                        for c in range(NQC):
                            cs = slice(c * 512, (c + 1) * 512)
                            ps = qkps.tile([128, 512], F32, tag="qkps", name="qkps")
                            for kt in range(NKT):
                                nc.tensor.matmul(ps, wt[:, kt, :], h_bf[kt][:, cs],
                                                 start=(kt == 0),
                                                 stop=(kt == NKT - 1))
                            nc.scalar.activation(qk_sb[f][:, cs], ps, AF.Identity,
                                                 bias=qkb_sb[:, f:f + 1])

                    # ---------- V natural [tokens, 12, 65] bf16 (aug ones) ------
                    v_aug = [lnpool.tile([128, 12, 65], BF16, tag=f"vaug{i}", name=f"vaug{i}")
                             for i in range(NTT)]
                    vw_sb = [wpool.tile([128, D], BF16, tag=f"vw{i}", name=f"vw{i}", bufs=1)
                             for i in range(NKT)]
                    for kt in range(NKT):
                        nc.sync.dma_start(vw_sb[kt],
                                          vw_d[l][kt * 128:(kt + 1) * 128, :])
                    for tt in range(NTT):
                        nc.vector.memset(v_aug[tt][:, :, 64:65], 1.0)
                        for vc in range(2):
                            vs = slice(vc * 384, (vc + 1) * 384)
                            ps = qkps.tile([128, 384], F32, tag="vps", name="vps")
                            for kt in range(NKT):
                                nc.tensor.matmul(
                                    ps, h_bf[kt][:, tt * 128:(tt + 1) * 128],
                                    vw_sb[kt][:, vs],
                                    start=(kt == 0), stop=False)
                            nc.tensor.matmul(ps, ones_row[:, 0:128],
                                             vbbf_sb[:, vs],
                                             start=False, stop=True)
                            nc.vector.tensor_copy(
                                v_aug[tt][:, vc * 6:(vc + 1) * 6, 0:64],
                                ps.rearrange("p (h d) -> p h d", d=64))

                # ---------- attention per head-pair ----------
                attoT = [lnpool.tile([128, T], BF16, tag=f"attoT{i}", name=f"attoT{i}")
                         for i in range(NKT)]
                with tc.tile_pool(name=f"sps{l}", bufs=2, space="PSUM") as sps, \
                     tc.tile_pool(name=f"ops{l}", bufs=1, space="PSUM") as ops, \
                     tc.tile_pool(name=f"tps{l}", bufs=1, space="PSUM") as tps, \
                     tc.tile_pool(name=f"attp{l}", bufs=1) as attp:
                    for pr in range(6):
                        attT = [[attp.tile([128, T], BF16, tag=f"attT{hh}_{kt}", name=f"attT{hh}_{kt}")
                                 for kt in range(NTT)] for hh in range(2)]
                        psT = tps.tile([128, T], BF16, tag="psT", name="psT")
                        for c in range(NQC):
                            cs = slice(c * 512, (c + 1) * 512)
                            nkt = 4 * (c + 1)
                            for kt in range(nkt):
                                ks = slice(kt * 128, (kt + 1) * 128)
                                pss = [None, None]
                                for hh in range(2):
                                    ps = sps.tile([128, 512], F32, tag=f"sps{hh}", name=f"sps{hh}")
                                    pss[hh] = ps
                                    hs = slice(hh * 64, hh * 64 + 64)
                                    nc.tensor.matmul(
                                        ps,
                                        qk_sb[6 + pr][hs, ks],   # kT [64,128]
                                        qk_sb[pr][hs, cs],       # qT [64,512]
                                        start=True, stop=True)
                                partial = (c == 0) or (kt >= 4)
                                for hh in range(2):
                                    dst = attT[hh][kt][:, cs]
                                    nc.scalar.activation(dst, pss[hh], AF.Exp,
                                                         scale=0.125)
                                    if partial:
                                        nc.vector.tensor_mul(
                                            dst, dst, mask_sb[:, kt % 4, :])
                        for hh in range(2):
                            h = 2 * pr + hh
                            for qt in range(NTT):
                                po = ops.tile([128, 65], F32, tag=f"ops{hh}", name=f"ops{hh}")
                                for kt in range(qt + 1):
                                    nc.tensor.matmul(
                                        po,
                                        attT[hh][kt][:, qt * 128:(qt + 1) * 128],
                                        v_aug[kt][:, h, :],
                                        start=(kt == 0), stop=(kt == qt))
                                r_sb = scratch.tile([128, 1], F32, tag="r_sb", name="r_sb")
                                ao = scratch.tile([128, 64], BF16, tag="ao", name="ao")
                                nc.vector.reciprocal(r_sb, po[:, 64:65])
                                nc.vector.tensor_scalar_mul(ao, po[:, 0:64], r_sb)
                                nc.tensor.transpose(
                                    psT[hh * 64:hh * 64 + 64,
                                        qt * 128:(qt + 1) * 128],
                                    ao, ident,
                                    tile_position=(0, hh * 64))
                        nc.vector.tensor_copy(attoT[pr], psT)

                # ---------- proj + residual ----------
                pw_sb = [wpool.tile([128, D], BF16, tag=f"pw{i}", name=f"pw{i}", bufs=1)
                         for i in range(NKT)]
                for kt in range(NKT):
                    nc.sync.dma_start(pw_sb[kt], pw_d[l][kt * 128:(kt + 1) * 128, :])
                with tc.tile_pool(name=f"pps{l}", bufs=4, space="PSUM") as pps:
                    for ot in range(NKT):
                        for c in range(NQC):
                            cs = slice(c * 512, (c + 1) * 512)
                            ps = pps.tile([128, 512], F32, tag="pps", name="pps")
                            for kt in range(NKT):
                                nc.tensor.matmul(
                                    ps, pw_sb[kt][:, ot * 128:(ot + 1) * 128],
                                    attoT[kt][:, cs],
                                    start=(kt == 0), stop=(kt == NKT - 1))
                            nc.vector.scalar_tensor_tensor(
                                xt[ot][:, cs], ps, pb_sb[:, ot:ot + 1],
                                xt[ot][:, cs], op0=ALU.add, op1=ALU.add)

                # ---------- LN2 + MLP (token-chunked hidden) ----------
                h2in = [lnpool.tile([128, T], BF16, tag=f"hbf{i}", name=f"hbf{i}")
                        for i in range(NKT)]
                _ln_phase(tc, nc, f"l{l}b", xt, ln_d[l][2], ln_d[l][3],
                          h2in, small, scratch)

                with tc.tile_pool(name=f"mlpps{l}", bufs=3, space="PSUM") as mlpps, \
                     tc.tile_pool(name=f"h2p{l}", bufs=1) as h2p:
                    for c in range(NQC):
                        cs = slice(c * 512, (c + 1) * 512)
                        h2c = [h2p.tile([128, 512], BF16, tag=f"h2c{f}", name=f"h2c{f}")
                               for f in range(24)]
                        for f in range(24):
                            wt = wpool.tile([128, NKT, 128], BF16, tag="fcw_t", name="fcw_t")
                            nc.sync.dma_start(
                                wt, fcw_d[l][:, f * 128:(f + 1) * 128]
                                .rearrange("(t p) f -> p t f", p=128))
                            ps = mlpps.tile([128, 512], F32, tag="fcps", name="fcps")
                            for kt in range(NKT):
                                nc.tensor.matmul(ps, wt[:, kt, :], h2in[kt][:, cs],
                                                 start=(kt == 0),
                                                 stop=(kt == NKT - 1))
                            nc.scalar.activation(h2c[f], ps, AF.Gelu_apprx_tanh,
                                                 bias=fcb_sb[:, f:f + 1])
                        for ot in range(NKT):
                            wt = wpool.tile([128, 24, 128], BF16, tag="fc2w_t", name="fc2w_t", bufs=2)
                            nc.sync.dma_start(
                                wt, fc2w_d[l][:, ot * 128:(ot + 1) * 128]
                                .rearrange("(t p) f -> p t f", p=128))
                            ps = mlpps.tile([128, 512], F32, tag="fc2ps", name="fc2ps")
                            for kt in range(24):
                                nc.tensor.matmul(ps, wt[:, kt, :], h2c[kt],
                                                 start=(kt == 0), stop=(kt == 23))
                            nc.vector.scalar_tensor_tensor(
                                xt[ot][:, cs], ps, fc2b_sb[:, ot:ot + 1],
                                xt[ot][:, cs], op0=ALU.add, op1=ALU.add)

        # ---------- final LN + lm_head ----------
        with ExitStack() as fctx:
            lnpool = fctx.enter_context(tc.tile_pool(name="lnfp", bufs=1))
            biasp = fctx.enter_context(tc.tile_pool(name="biasf", bufs=1))
            small = fctx.enter_context(tc.tile_pool(name="smallf", bufs=2))
            scratch = fctx.enter_context(tc.tile_pool(name="scrf", bufs=3))
            xf_bf = [lnpool.tile([128, T], BF16, tag=f"xf{i}", name=f"xf{i}") for i in range(NKT)]
            _ln_phase(tc, nc, "lf", xt, lnf_d[0], lnf_d[1],
                      xf_bf, small, scratch)

            with tc.tile_pool(name="lmw", bufs=3) as lmwp, \
                 tc.tile_pool(name="lmps", bufs=4, space="PSUM") as lmps, \
                 tc.tile_pool(name="lmev", bufs=4) as lmev:
                for vc in range(NVC):
                    wt = lmwp.tile([128, NKT, 512], BF16, tag="lmw_t", name="lmw_t")
                    nc.sync.dma_start(
                        wt, lmw_d[:, vc * 512:(vc + 1) * 512]
                        .rearrange("(t p) v -> p t v", p=128))
                    for tt in range(NTT):
                        ps = lmps.tile([128, 512], F32, tag="lmps", name="lmps")
                        for kt in range(NKT):
                            nc.tensor.matmul(
                                ps, xf_bf[kt][:, tt * 128:(tt + 1) * 128],
                                wt[:, kt, :],
                                start=(kt == 0), stop=(kt == NKT - 1))
                        ev = lmev.tile([128, 512], F16, tag="lmev", name="lmev")
                        if tt % 2 == 0:
                            nc.scalar.copy(ev, ps)
                        else:
                            nc.vector.tensor_copy(ev, ps)
                        nc.sync.dma_start(
                            out_d[tt * 128:(tt + 1) * 128,
                                  vc * 512:(vc + 1) * 512], ev)
    nc.finalize()
    return nc


_NC_CACHE = None


def _get_nc():
    global _NC_CACHE
    if _NC_CACHE is None:
        _NC_CACHE = build_bass()
    return _NC_CACHE


def make_in_maps(idx, layer_num, wte, wpe, ln1_g, ln1_b, attn_w, attn_b, proj_w,
                 proj_b, ln2_g, ln2_b, fc_w, fc_b, fc2_w, fc2_b, lnf_g, lnf_b, lm_w):
    bf = ml_dtypes.bfloat16
    idx = np.asarray(idx)
    f32 = np.float32
    wte = np.asarray(wte, f32)
    wpe = np.asarray(wpe, f32)
    x0 = wte[idx] + wpe[:T]                      # [B,T,D] fp32 host embedding

    qkw = np.ascontiguousarray(np.asarray(attn_w, f32)[:, :, :2 * D]).astype(bf)
    vw = np.ascontiguousarray(np.asarray(attn_w, f32)[:, :, 2 * D:]).astype(bf)
    pw = np.asarray(proj_w, f32).astype(bf)
    fcw = np.asarray(fc_w, f32).astype(bf)
    fc2w = np.asarray(fc2_w, f32).astype(bf)
    qkb = np.ascontiguousarray(np.asarray(attn_b, f32)[:, :2 * D])
    vb = np.ascontiguousarray(np.asarray(attn_b, f32)[:, 2 * D:]).astype(bf)
    lnp = np.stack([np.asarray(ln1_g, f32), np.asarray(ln1_b, f32),
                    np.asarray(ln2_g, f32), np.asarray(ln2_b, f32)], axis=1).astype(bf)
    lnf = np.stack([np.asarray(lnf_g, f32), np.asarray(lnf_b, f32)], axis=0).astype(bf)

    lmw_pad = np.zeros((D, VPAD), f32)
    lmw_pad[:, :V] = np.asarray(lm_w, f32)
    lmw_bf = lmw_pad.astype(bf)

    # causal mask blocks: mask[j][kk, qq] = (128*j + kk) <= qq
    jj = np.arange(4)[:, None, None] * 128 + np.arange(128)[None, :, None]
    qq = np.arange(512)[None, None, :]
    mask = (jj <= qq).astype(bf)

    in_maps = []
    for core in range(8):
        b = core // 2
        vs = (core % 2) * VSH
        in_maps.append(dict(
            xT=np.ascontiguousarray(x0[b].T),
            qkw=qkw, vw=vw, pw=pw, fcw=fcw, fc2w=fc2w,
            qkb=qkb, vb=vb, pb=np.asarray(proj_b, f32),
            fcb=np.asarray(fc_b, f32), fc2b=np.asarray(fc2_b, f32),
            lnp=lnp, lnf=lnf, mask=mask,
            lmw=np.ascontiguousarray(lmw_bf[:, vs:vs + VSH]),
        ))
    return in_maps


def kernel(**inputs):
    global LAST_RESULT
    in_maps = make_in_maps(**inputs)
    nc = _get_nc()
    res = run_bass_kernel_spmd(nc, in_maps, core_ids=list(range(8)), trace=TRACE)
    LAST_RESULT = res

    logits = np.empty((B, T, V), np.float32)
    for b in range(B):
        lo = res.results[2 * b]["out"].astype(np.float32)
        hi = res.results[2 * b + 1]["out"].astype(np.float32)
        logits[b, :, :VSH] = lo
        logits[b, :, VSH:] = hi[:, :V - VSH]
    return logits

